# revision 1
# baseline (speedup 1.0000x reference)
"""Trainium2 Bass kernel for nn_ConvWindowAttention.

Reference computation (per position n, fp32):
    qkv = x @ qkv_w.T + qkv_b                     # [B,N,3C], C=512
    q,k,v = split(qkv) -> [B,N,H,hd], H=8, hd=64
    scores = einsum('bnhd,bngd->bnhg', q, k) / sqrt(hd)   # per-position HxH
    attn = softmax(scores, -1)
    att  = einsum('bnhg,bngd->bnhd', attn, v)
    y    = att.transpose(0,2,1,3).reshape(B,N,C)  # axis-interleaving reshape
    out  = y @ out_w.T + out_b

Sharding: 8 cores, each takes (b, n-half) = 2048 positions. The interleaved
reshape maps attention outputs of 8 consecutive positions into single rows of
y, so a 2048-position shard is fully self-contained (no cross-core traffic).

Per-core device pipeline (16 tiles of 128 positions):
  GEMM1 (PE, f32r)  -> psum qkv -> ACT casts to fp16 (q scaled by 1/sqrt(hd),
  v emitted in (d,g)-transposed layout via host-permuted weight columns)
  scores/softmax/att@v on DVE in fp16 (broadcast muls + halving-tree reduces)
  y permute (SBUF->SBUF DMA), PE transpose -> yT, GEMM2 (PE, fp16) + bias,
  psum -> DRAM.
Biases are applied as K=1 matmuls (ones-row x bias-row) accumulated into the
same PSUM group, costing no vector-engine work.
"""

import numpy as np

B, N, C = 4, 4096, 512
H, HD = 8, 64
NCORES = 8
R = (B * N) // NCORES          # rows (positions) per core = 2048
P = 128
TILES = R // P                 # 16
O1 = 3 * C                     # 1536

_NC_CACHE = {}


def _host_prep(x, qkv_w, qkv_b, out_w, out_b):
    """Shard x and pre-layout weights. Returns per-core input maps."""
    x = np.ascontiguousarray(np.asarray(x, dtype=np.float32))
    qkv_w = np.asarray(qkv_w, dtype=np.float32)
    qkv_b = np.asarray(qkv_b, dtype=np.float32)
    out_w = np.asarray(out_w, dtype=np.float32)
    out_b = np.asarray(out_b, dtype=np.float32)

    w1 = np.ascontiguousarray(qkv_w.T)            # [C, 3C]
    # v block columns permuted (g,d) -> (d,g) so the attn@v product reads v
    # with g innermost (keeps the DVE 2x perf mode on both operands).
    vblk = w1[:, 2 * C:].reshape(C, H, HD)
    w1 = np.concatenate([w1[:, : 2 * C], vblk.transpose(0, 2, 1).reshape(C, C)], axis=1)
    w1 = np.ascontiguousarray(w1).astype(np.float16)

    b1 = qkv_b.copy()
    vb = b1[2 * C:].reshape(H, HD)
    b1 = np.concatenate([b1[: 2 * C], vb.T.reshape(C)])
    b1 = b1.reshape(1, O1).astype(np.float16)

    w2 = np.ascontiguousarray(out_w.T).astype(np.float16)   # [C, C]
    b2 = out_b.reshape(1, C).astype(np.float16)

    in_maps = []
    for core in range(NCORES):
        b, half = core // 2, core % 2
        n0 = half * R
        xs = np.ascontiguousarray(x[b, n0:n0 + R, :].T.astype(np.float16))
        in_maps.append({"xT": xs, "w1": w1, "b1": b1, "w2": w2, "b2": b2})
    return in_maps


def _host_gather(results, dtype):
    """Reassemble full [B,N,C] output from per-core compact [R,C] outputs."""
    out = np.empty((B, N, C), dtype=np.float32)
    for core in range(NCORES):
        b, half = core // 2, core % 2
        dev = results[core]["out"]                           # [2048, 512]
        # device row r = t*128 + h*16 + k  ->  n' = h*512 + half*256 + 16t + k
        arr = dev.reshape(TILES, H, 16, C).transpose(1, 0, 2, 3).reshape(H, 256, C)
        for h in range(H):
            lo = h * 512 + half * 256
            out[b, lo:lo + 256, :] = arr[h]
    return out.astype(dtype, copy=False)


def _emit(tc, aps, reps=1):
    import concourse.bass as bass
    import concourse.mybir as mybir

    nc = tc.nc
    f32, f16 = mybir.dt.float32, mybir.dt.float16
    f32r = mybir.dt.float32r
    Exp = mybir.ActivationFunctionType.Exp

    xT, w1, b1, w2, b2, out = (aps[k] for k in ("xT", "w1", "b1", "w2", "b2", "out"))
    y_dram = nc.dram_tensor("y_scratch", [R, C], f16).ap()

    import contextlib
    ctx = contextlib.ExitStack()
    with ctx:
        consts = ctx.enter_context(tc.tile_pool(name="consts", bufs=1))
        xt_pool = ctx.enter_context(tc.tile_pool(name="xt", bufs=4))
        qkv_sb = ctx.enter_context(tc.tile_pool(name="qkv_sb", bufs=3))
        prod_pool = ctx.enter_context(tc.tile_pool(name="prod", bufs=3))
        tree_pool = ctx.enter_context(tc.tile_pool(name="tree", bufs=3))
        soft_pool = ctx.enter_context(tc.tile_pool(name="soft", bufs=3))
        y_pool = ctx.enter_context(tc.tile_pool(name="y", bufs=3))
        ps_qkv = ctx.enter_context(tc.tile_pool(name="ps_qkv", bufs=2, space="PSUM"))
        ps_yt = ctx.enter_context(tc.tile_pool(name="ps_yt", bufs=1, space="PSUM"))
        ps_o2 = ctx.enter_context(tc.tile_pool(name="ps_o2", bufs=1, space="PSUM"))

        # ---- constants ----
        w1_sb = []
        for kk in range(4):
            wt = consts.tile([P, O1], f16, tag=f"w1_{kk}")
            nc.sync.dma_start(out=wt[:], in_=w1[kk * P:(kk + 1) * P, :])
            w1_sb.append(wt)
        w2_sb = []
        for kk in range(4):
            wt = consts.tile([P, C], f16, tag=f"w2_{kk}")
            nc.sync.dma_start(out=wt[:], in_=w2[kk * P:(kk + 1) * P, :])
            w2_sb.append(wt)
        b1_sb = consts.tile([1, O1], f16, tag="b1")
        nc.sync.dma_start(out=b1_sb[:], in_=b1)
        b2_sb = consts.tile([1, C], f16, tag="b2")
        nc.sync.dma_start(out=b2_sb[:], in_=b2)
        ones1 = consts.tile([1, P], f16, tag="ones1")
        nc.vector.memset(ones1[:], 1.0)
        ident = consts.tile([P, P], f16, tag="ident")
        from concourse.masks import make_identity
        make_identity(nc, ident[:])

        scale = HD ** -0.5

        def body():
            for t in range(TILES):
                _tile(t)

        def _tile(t):
            tsl = slice(t * P, (t + 1) * P)

            # ---- load xT tile (4 k-blocks side by side), one DMA ----
            xtt = xt_pool.tile([P, 4 * P], f16, tag="xt")
            src = xT[:, tsl].rearrange("(kb p) c -> p kb c", kb=4)
            nc.sync.dma_start(out=xtt[:].rearrange("p (kb c) -> p kb c", kb=4),
                                in_=src)

            # ---- GEMM1: qkv[n, o] ----
            qkv_ps = ps_qkv.tile([P, O1], f32, tag="qkv")
            for kk in range(4):
                for oc in range(3):
                    osl = slice(oc * C, (oc + 1) * C)
                    nc.tensor.matmul(
                        qkv_ps[:, osl],
                        lhsT=xtt[:, kk * P:(kk + 1) * P],
                        rhs=w1_sb[kk][:, osl],
                        start=(kk == 0),
                        stop=False,
                    )
            for oc in range(3):
                osl = slice(oc * C, (oc + 1) * C)
                nc.tensor.matmul(
                    qkv_ps[:, osl], lhsT=ones1[:], rhs=b1_sb[:, osl],
                    start=False, stop=True,
                )

            # ---- casts to fp16 (ACT), q pre-scaled ----
            q_sb = qkv_sb.tile([P, C], f16, tag="q")
            nc.scalar.mul(q_sb[:], qkv_ps[:, 0:C], scale)
            k_sb = qkv_sb.tile([P, C], f16, tag="k")
            nc.scalar.copy(k_sb[:], qkv_ps[:, C:2 * C])
            v_sb = qkv_sb.tile([P, C], f16, tag="v")       # (d,g) layout
            nc.scalar.copy(v_sb[:], qkv_ps[:, 2 * C:3 * C])

            # ---- scores products: [p, h, g, d] ----
            prod = prod_pool.tile([P, H * H * HD], f16, tag="prod")
            q_ap = q_sb[:].rearrange("p (h d) -> p h d", h=H).unsqueeze(2) \
                          .broadcast_to((P, H, H, HD))
            k_ap = k_sb[:].rearrange("p (g d) -> p g d", g=H).unsqueeze(1) \
                          .broadcast_to((P, H, H, HD))
            p_ap = prod[:].rearrange("p (h g d) -> p h g d", h=H, g=H)
            nc.vector.tensor_mul(p_ap, q_ap, k_ap)

            # ---- reduce over d (halving tree, innermost contiguous) ----
            lv = prod[:].rearrange("p (hg d) -> p hg d", hg=H * H)
            widths = [32, 16, 8]
            cur = lv
            for w in widths:
                nt = tree_pool.tile([P, H * H, w], f16, tag=f"sc_t{w}")
                nc.vector.tensor_add(nt[:], cur[:, :, 0:w], cur[:, :, w:2 * w])
                cur = nt
            scores = tree_pool.tile([P, H * H], f32, tag="scores")
            nc.vector.reduce_sum(scores[:].unsqueeze(2), cur[:],
                                 axis=mybir.AxisListType.X)

            # ---- softmax over g (no max-sub: |scores| <~ 8, exp fits fp16) ----
            attn = soft_pool.tile([P, H * H], f16, tag="attn")
            nc.scalar.activation(attn[:], scores[:], Exp)

            a3 = attn[:].rearrange("p (h g) -> p h g", h=H)
            ssum = soft_pool.tile([P, H], f32, tag="ssum")
            nc.vector.reduce_sum(ssum[:].unsqueeze(2), a3, axis=mybir.AxisListType.X)

            rec = soft_pool.tile([P, H], f32, tag="rec")
            nc.vector.reciprocal(rec[:], ssum[:])
            attn_n = soft_pool.tile([P, H * H], f16, tag="attn_n")
            rec_b = rec[:].unsqueeze(2).broadcast_to((P, H, H))
            nc.vector.tensor_mul(
                attn_n[:].rearrange("p (h g) -> p h g", h=H), a3, rec_b)

            # ---- att@v products: [p, h, d, g] (g innermost on both) ----
            prod2 = prod_pool.tile([P, H * HD * H], f16, tag="prod")
            a_ap = attn_n[:].rearrange("p (h g) -> p h g", h=H).unsqueeze(2) \
                            .broadcast_to((P, H, HD, H))
            v_ap = v_sb[:].rearrange("p (d g) -> p d g", d=HD).unsqueeze(1) \
                          .broadcast_to((P, H, HD, H))
            p2_ap = prod2[:].rearrange("p (h d g) -> p h d g", h=H, d=HD)
            nc.vector.tensor_mul(p2_ap, a_ap, v_ap)

            # ---- reduce over g ----
            u0 = prod2[:].rearrange("p (hd g) -> p hd g", hd=H * HD)
            u1 = tree_pool.tile([P, H * HD, 4], f16, tag="u1")
            nc.vector.tensor_add(u1[:], u0[:, :, 0:4], u0[:, :, 4:8])
            u2 = tree_pool.tile([P, H * HD, 2], f16, tag="u2")
            nc.vector.tensor_add(u2[:], u1[:, :, 0:2], u1[:, :, 2:4])
            y = y_pool.tile([P, C], f16, tag="y")          # [(k j), (h d)]
            nc.vector.tensor_add(y[:].unsqueeze(2), u2[:, :, 0:1], u2[:, :, 1:2])

            # ---- permute y -> y_perm [(h k), (j d)] via DRAM roundtrip ----
            # store raw [n_local, (h d)]; the load gathers [(h k), (j d)] in
            # one 3-dim AP: enumerate (h, k, j, d); (k, j) merge since the raw
            # j-stride (C) times j-count (8) equals the k-stride (8C).
            nc.sync.dma_start(out=y_dram[t * P:(t + 1) * P, :], in_=y[:])
            y_perm = y_pool.tile([P, C], f16, tag="y_perm")
            ybase = y_dram[:]
            src = bass.AP(
                tensor=ybase.tensor, offset=ybase.offset + t * P * C,
                ap=[[HD, H], [C, P], [1, HD]],
            )
            nc.sync.dma_start(out=y_perm[:], in_=src)

            # ---- transpose y_perm -> yT (PE), copy to SBUF ----
            yt_ps = ps_yt.tile([P, C], f16, tag="yt")
            for J in range(4):
                jsl = slice(J * P, (J + 1) * P)
                nc.tensor.transpose(yt_ps[:, jsl], y_perm[:, jsl], ident[:])
            yt_sb = y_pool.tile([P, C], f16, tag="yt_sb")
            nc.scalar.copy(yt_sb[:], yt_ps[:])

            # ---- GEMM2 + bias ----
            o_ps = ps_o2.tile([P, C], f32, tag="o2")
            for J in range(4):
                jsl = slice(J * P, (J + 1) * P)
                nc.tensor.matmul(o_ps[:], lhsT=yt_sb[:, jsl], rhs=w2_sb[J][:],
                                 start=(J == 0), stop=False)
            nc.tensor.matmul(o_ps[:], lhsT=ones1[:], rhs=b2_sb[:],
                             start=False, stop=True)

            o_sb = y_pool.tile([P, C], f32, tag="o_sb")
            nc.scalar.copy(o_sb[:], o_ps[:])
            nc.sync.dma_start(out=out[tsl, :], in_=o_sb[:])

        if reps == 1:
            body()
        else:
            with tc.For_i(0, reps, 1):
                body()


def _build_nc(reps=1):
    import concourse.bacc as bacc
    import concourse.mybir as mybir
    import concourse.tile as tile

    f32, f16 = mybir.dt.float32, mybir.dt.float16
    nc = bacc.Bacc("TRN2", target_bir_lowering=False, debug=False)
    aps = {
        "xT": nc.dram_tensor("xT", [C, R], f16, kind="ExternalInput").ap(),
        "w1": nc.dram_tensor("w1", [C, O1], f16, kind="ExternalInput").ap(),
        "b1": nc.dram_tensor("b1", [1, O1], f16, kind="ExternalInput").ap(),
        "w2": nc.dram_tensor("w2", [C, C], f16, kind="ExternalInput").ap(),
        "b2": nc.dram_tensor("b2", [1, C], f16, kind="ExternalInput").ap(),
        "out": nc.dram_tensor("out", [R, C], f32, kind="ExternalOutput").ap(),
    }
    with tile.TileContext(nc) as tc:
        _emit(tc, aps, reps=reps)
    nc.compile()
    return nc


def _get_nc(reps=1):
    if reps not in _NC_CACHE:
        _NC_CACHE[reps] = _build_nc(reps)
    return _NC_CACHE[reps]


def kernel(x, qkv_w, qkv_b, out_w, out_b):
    from concourse.bass_utils import run_bass_kernel_spmd

    in_dtype = np.asarray(x).dtype
    in_maps = _host_prep(x, qkv_w, qkv_b, out_w, out_b)
    nc = _get_nc()
    res = run_bass_kernel_spmd(nc, in_maps, list(range(NCORES)))
    return _host_gather(res.results, in_dtype)

